# revision 1
# baseline (speedup 1.0000x reference)
"""Trainium2 distributed kernel: pct-permute + GroupNorm(1 group) + residual + SE block.

Sharding: spatial over H (112 rows -> 14 rows per core, 8 cores).
 - pct (batch-block channel permute) is fully local under spatial sharding.
 - GroupNorm mean/var and SE's global-average-pool both reduce over
   (C,H,W) / (H,W); per-(sample,channel) sums and sums-of-squares of the
   *permuted* tensor are computed locally and combined with ONE small
   AllReduce ([128, 48] f32 = 24.5 KB), because the GN normalization is
   affine per (sample, channel): GAP(z) is derivable from per-channel
   sums of x.
 - Everything else (SE matmuls, sigmoid, final scaling) is computed
   redundantly per core on tiny tensors.

Pipeline per core (x shard [8, 384, 14, 112] stays resident in SBUF):
   load -> bn_stats (per-channel sum/sumsq) -> AllReduce -> tiny SE math
        -> fused affine+residual+scale output pass -> store.
"""

import sys

if "/opt/trn_rl_repo" not in sys.path:
    sys.path.insert(0, "/opt/trn_rl_repo")

import numpy as np

N, C, H, W = 8, 384, 112, 112
HID = C // 16  # 24
NCORES = 8
HS = H // NCORES  # 14
SP = HS * W  # 1568 spatial elements per shard plane
DP = (C // 3) // N  # 16
M = N * DP  # 128 permuted channels
CT = C // 128  # 3 channel tiles
NPIX = H * W  # 12544
CNT = C * NPIX
GN_EPS = 1e-5

_compiled = {}


def _build():
    import concourse.bass as bass
    import concourse.bacc as bacc
    import concourse.mybir as mybir
    import concourse.tile as tile

    fp32 = mybir.dt.float32
    bf16 = mybir.dt.bfloat16
    Alu = mybir.AluOpType
    Act = mybir.ActivationFunctionType
    Ax = mybir.AxisListType

    nc = bacc.Bacc("TRN2", target_bir_lowering=False, debug=False, num_devices=NCORES)

    xs = nc.dram_tensor("x", [N, C, HS, W], fp32, kind="ExternalInput").ap()
    gnw = nc.dram_tensor("gnw", [C], fp32, kind="ExternalInput").ap()
    gnb = nc.dram_tensor("gnb", [C], fp32, kind="ExternalInput").ap()
    w1d = nc.dram_tensor("w1", [C, HID], fp32, kind="ExternalInput").ap()
    w2d = nc.dram_tensor("w2", [HID, C], fp32, kind="ExternalInput").ap()
    bandsd = nc.dram_tensor("bands", [128, 480], bf16, kind="ExternalInput").ap()
    outd = nc.dram_tensor("out", [N, C, HS, W], fp32, kind="ExternalOutput").ap()

    c1 = 1.0 / NPIX  # per-channel mean scale
    cC = 1.0 / CNT  # per-sample (C,H,W) mean scale

    with tile.TileContext(nc) as tc:
        with (
            tc.tile_pool(name="xp", bufs=1) as xp,
            tc.tile_pool(name="sp", bufs=1) as spool,
            tc.tile_pool(name="ps", bufs=1, space="PSUM") as ps,
            tc.tile_pool(name="dram", bufs=1, space="DRAM") as dram,
        ):
            # ---- resident x tiles + permuted ct0 tiles ----
            xt = {}
            for t in range(CT):
                for j in range(N):
                    xt[t, j] = xp.tile([128, SP], fp32, tag=f"x_{t}_{j}", name=f"x_{t}_{j}")
            pct = [xp.tile([128, SP], bf16, tag=f"p_{j}", name=f"p_{j}") for j in range(N)]
            xb = [xp.tile([128, SP], bf16, tag=f"xb_{j}", name=f"xb_{j}") for j in range(N)]

            # ---- small tiles ----
            ST = spool.tile([128, 24, 4, 6], fp32, tag="ST", name="ST")  # raw bn_stats
            STAT = spool.tile([128, 56], fp32, tag="STAT", name="STAT")  # local sums|sumsqs
            GS = spool.tile([128, 56], fp32, tag="GS", name="GS")  # allreduced
            tmp1 = spool.tile([128, 24, 4, 2], fp32, tag="tmp1", name="tmp1")
            tmp2 = spool.tile([128, 24, 4, 2], fp32, tag="tmp2", name="tmp2")
            ones_col = spool.tile([1, 128], fp32, tag="ones_col", name="ones_col")
            ones128 = spool.tile([128, 1], fp32, tag="ones128", name="ones128")
            Mrow = spool.tile([1, 16], fp32, tag="Mrow", name="Mrow")  # mu | inv
            rsum = spool.tile([1, 8], fp32, tag="rsum", name="rsum")
            e2r = spool.tile([1, 8], fp32, tag="e2r", name="e2r")
            varr = spool.tile([1, 8], fp32, tag="varr", name="varr")
            sdr = spool.tile([1, 8], fp32, tag="sdr", name="sdr")
            MB = spool.tile([128, 16], fp32, tag="MB", name="MB")  # mu_b | inv_b
            gw = spool.tile([128, CT], fp32, tag="gw", name="gw")
            gb = spool.tile([128, CT], fp32, tag="gb", name="gb")
            w1s = spool.tile([128, CT, HID], fp32, tag="w1s", name="w1s")
            w2s = spool.tile([HID, C], fp32, tag="w2s", name="w2s")
            uts = spool.tile([HID, N], fp32, tag="uts", name="uts")
            gts = [spool.tile([128, 8], fp32, tag=f"g_{t}", name=f"g_{t}") for t in range(CT)]
            sct = [spool.tile([128, 8], fp32, tag=f"s_{t}", name=f"s_{t}") for t in range(CT)]
            Ac = [spool.tile([128, 8], fp32, tag=f"A_{t}", name=f"A_{t}") for t in range(CT)]
            Bc = [spool.tile([128, 8], fp32, tag=f"B_{t}", name=f"B_{t}") for t in range(CT)]
            sF = [spool.tile([128, 8], fp32, tag=f"sF_{t}", name=f"sF_{t}") for t in range(CT)]
            bF = [spool.tile([128, 8], fp32, tag=f"bF_{t}", name=f"bF_{t}") for t in range(CT)]
            t8a = spool.tile([128, 8], fp32, tag="t8a", name="t8a")
            t8b = spool.tile([128, 8], fp32, tag="t8b", name="t8b")
            epsc = spool.tile([1, 1], fp32, tag="epsc", name="epsc")
            EVb = spool.tile([128, 240], bf16, tag="EVb", name="EVb")
            ODb = spool.tile([128, 240], bf16, tag="ODb", name="ODb")
            Trow = spool.tile([1, 56], fp32, tag="Trow", name="Trow")
            ACCQ = spool.tile([128, 5, 4], fp32, tag="ACCQ", name="ACCQ")

            psSM = ps.tile([128, 392], fp32, tag="pp0", name="psSM", bufs=2)
            psT = psSM[0:1, 0:56]
            psB = psSM[:, 56:72]
            psU = psSM[0:HID, 72:80]
            psS = [psSM[:, 80 + 8 * t : 88 + 8 * t] for t in range(CT)]
            CH = SP // 4  # 392

            arin = dram.tile([128, 56], fp32, name="arin")
            arout = dram.tile([128, 56], fp32, name="arout")
            brin = dram.tile([1, 1], fp32, name="brin")
            brout = dram.tile([1, 1], fp32, name="brout")

            # startup barrier: absorbs cross-core launch skew before real work
            nc.gpsimd.memset(epsc[:], GN_EPS)
            nc.gpsimd.dma_start(brin[:], epsc[:])
            nc.gpsimd.collective_compute(
                "AllReduce",
                Alu.add,
                replica_groups=[list(range(NCORES))],
                ins=[brin.opt()],
                outs=[brout.opt()],
            )

            # ---- constants / weights (SWDGE queue) ----
            nc.gpsimd.memset(ones_col[:], 1.0)
            nc.gpsimd.memset(ones128[:], 1.0)
            nc.gpsimd.dma_start(gw[:], gnw.rearrange("(t c) -> c t", c=128))
            nc.gpsimd.dma_start(gb[:], gnb.rearrange("(t c) -> c t", c=128))
            nc.gpsimd.dma_start(w1s[:], w1d.rearrange("(t c) h -> c t h", c=128))
            nc.gpsimd.dma_start(w2s[:], w2d[:])
            # EV/OD: periodic banded permutation weights, precomputed host-side
            # EVb[k, f] = 1 iff (k%32)<16 and f == 112 + (k%32)
            # ODb[k, f] = 1 iff (k%32)>=16 and f == 96 + (k%32)
            nc.gpsimd.dma_start(EVb[:], bandsd[:, 0:240])
            nc.gpsimd.dma_start(ODb[:], bandsd[:, 240:480])

            # second tiny collective: further warms the mesh path off the
            # critical path (first collective pays one-time setup)
            nc.gpsimd.dma_start(brin[:], epsc[:])
            nc.gpsimd.collective_compute(
                "AllReduce",
                Alu.add,
                replica_groups=[list(range(NCORES))],
                ins=[brin.opt()],
                outs=[brout.opt()],
            )

            # ---- load x shard: ct0 first (pct build + ACT sums), then ct1/ct2 ----
            for t in (0, 1, 2):
                for j in range(N):
                    nc.sync.dma_start(
                        xt[t, j][:],
                        xs[j, t * 128 : (t + 1) * 128].rearrange("c h w -> c (h w)"),
                    )

            # bf16 copies of ct0 tiles (TensorEngine runs the permute in bf16).
            # On DVE: runs before the ct1/ct2 bn_stats sources even arrive,
            # unblocking the PE permute ~20us earlier than via ScalarE.
            for j in range(N):
                nc.vector.tensor_copy(xb[j][:], xt[0, j][:])


            # ---- build permuted tiles on the (otherwise idle) TensorEngine:
            # pct[j][16i+r, :] = x0[i][16j+r, :].  j handled in 32-row pairs
            # (jl=0 even j via EVb band, jl=1 odd j via ODb band).  The q
            # (row-quadrant) loop is innermost so consecutive matmuls target
            # different PE row groups -> LDWEIGHTS overlaps the prior MATMUL.
            for jl, band in ((0, EVb), (1, ODb)):
                for ch in range(4):
                    pp = [
                        ps.tile(
                            [128, CH], fp32, tag=f"pp{q}",
                            name=f"pp{q}_{jl}_{ch}", bufs=2,
                        )
                        for q in range(4)
                    ]
                    for i in range(N):
                        for q in range(4):
                            nc.tensor.matmul(
                                pp[q][:],
                                band[32 * q : 32 * (q + 1), 112 - 16 * i : 240 - 16 * i],
                                xb[i][32 * q : 32 * (q + 1), ch * CH : (ch + 1) * CH],
                                start=(i == 0),
                                stop=(i == N - 1),
                                tile_position=(32 * q, 0),
                            )
                    for q in range(4):
                        nc.scalar.activation(
                            pct[2 * q + jl][:, ch * CH : (ch + 1) * CH], pp[q][:],
                            Act.Copy,
                        )

            # ---- local stats: bn_stats per (tile-slot); slots: ct1 | ct2 | pct ----
            def slot_src(s):
                if s < 8:
                    return xt[1, s]
                if s < 16:
                    return xt[2, s - 8]
                return pct[s - 16]

            # bn_stats order matched to arrival: x1 loads land first, the PE
            # finishes even-j then odd-j pct tiles mid-phase, x2 loads last.
            order = (
                list(range(0, 8))
                + [16 + j for j in (0, 2, 4, 6)]
                + [16 + j for j in (1, 3, 5, 7)]
                + list(range(8, 16))
            )
            for s in order:
                src = slot_src(s)
                for ch in range(4):
                    nc.vector.bn_stats(
                        ST[:, s, ch, :], src[:, ch * CH : (ch + 1) * CH]
                    )

            # convert (count, mean, count*var) x (even, odd) -> sums | sumsqs
            def convert(sts, t1, t2, sum_cols, sq_cols, nsl):
                cnts = sts[:, :, :, 0::3]
                means = sts[:, :, :, 1::3]
                cvars = sts[:, :, :, 2::3]
                nc.vector.tensor_tensor(t1, cnts, means, Alu.mult)  # c*m
                nc.vector.tensor_reduce(
                    sum_cols, t1.rearrange("c a b e -> c a (b e)"), Ax.X, Alu.add
                )
                nc.vector.tensor_tensor(t2, means, means, Alu.mult)  # m^2
                nc.vector.tensor_tensor(t2, t2, cnts, Alu.mult)  # c*m^2
                nc.vector.tensor_tensor(t2, t2, cvars, Alu.add)  # + c*var
                nc.vector.tensor_reduce(
                    sq_cols, t2.rearrange("c a b e -> c a (b e)"), Ax.X, Alu.add
                )

            convert(
                ST[:, 0:16], tmp1[:, 0:16], tmp2[:, 0:16],
                STAT[:, 0:16], STAT[:, 16:32], 16,
            )
            convert(
                ST[:, 16:24], tmp1[:, 16:19 + 5], tmp2[:, 16:19 + 5],
                STAT[:, 32:40], STAT[:, 48:56], 8,
            )

            # un-permuted ct0 per-channel sums (for GAP shortcut term) on ScalarE
            # (in-place Copy is a data no-op; accum_out yields the row sum)
            for j in range(N):
                nc.scalar.activation(
                    xt[0, j][:], xt[0, j][:], Act.Copy,
                    accum_out=STAT[:, 40 + j : 41 + j],
                )

            # ---- AllReduce ----
            nc.sync.dma_start(arin[:], STAT[:])
            nc.gpsimd.collective_compute(
                "AllReduce",
                Alu.add,
                replica_groups=[list(range(NCORES))],
                ins=[arin.opt()],
                outs=[arout.opt()],
            )
            nc.sync.dma_start(GS[:], arout[:])

            # GS: c12 sums 0:16 | c12 sq 16:32 | pct sums 32:40 | X0 40:48 | pct sq 48:56
            Ssl = {1: GS[:, 0:8], 2: GS[:, 8:16], 0: GS[:, 32:40]}  # 0 == permuted!
            X0s = GS[:, 40:48]

            # ---- mu / var per sample ----
            nc.tensor.matmul(psT, ones128[:], GS[:], start=True, stop=True)
            nc.vector.tensor_copy(Trow[:], psT)
            nc.vector.tensor_tensor(rsum[:], Trow[:, 0:8], Trow[:, 8:16], Alu.add)
            nc.vector.tensor_tensor(rsum[:], rsum[:], Trow[:, 32:40], Alu.add)
            nc.vector.tensor_scalar(Mrow[:, 0:8], rsum[:], cC, None, Alu.mult)  # mu
            nc.vector.tensor_tensor(e2r[:], Trow[:, 16:24], Trow[:, 24:32], Alu.add)
            nc.vector.tensor_tensor(e2r[:], e2r[:], Trow[:, 48:56], Alu.add)
            nc.vector.tensor_scalar(e2r[:], e2r[:], cC, None, Alu.mult)  # E[y^2]
            nc.vector.tensor_tensor(varr[:], Mrow[:, 0:8], Mrow[:, 0:8], Alu.mult)
            nc.vector.tensor_tensor(varr[:], e2r[:], varr[:], Alu.subtract)
            nc.scalar.activation(sdr[:], varr[:], Act.Sqrt, bias=epsc[:, 0:1], scale=1.0)
            nc.vector.reciprocal(Mrow[:, 8:16], sdr[:])  # inv = rsqrt(var+eps)

            # broadcast mu|inv across partitions
            nc.tensor.matmul(psB, ones_col[:], Mrow[:], start=True, stop=True)
            nc.vector.tensor_copy(MB[:], psB)
            mu_b, inv_b = MB[:, 0:8], MB[:, 8:16]

            # ---- g (GAP of z) in [channel, sample] layout, then SE ----
            for t in range(CT):
                msrc = Ssl[t]
                mx = X0s if t == 0 else msrc
                g = gts[t]
                nc.vector.tensor_scalar(g[:], msrc, c1, None, Alu.mult)
                nc.vector.tensor_tensor(g[:], g[:], mu_b, Alu.subtract)
                nc.vector.tensor_tensor(g[:], g[:], inv_b, Alu.mult)
                nc.vector.tensor_scalar(
                    g[:], g[:], gw[:, t : t + 1], gb[:, t : t + 1], Alu.mult, Alu.add
                )
                nc.vector.tensor_scalar(t8a[:], mx, c1, None, Alu.mult)
                nc.vector.tensor_tensor(g[:], g[:], t8a[:], Alu.add)

            # uT = relu(w1^T @ g)  [HID, N]
            for t in range(CT):
                nc.tensor.matmul(
                    psU,
                    w1s[:, t, :],
                    gts[t][:],
                    start=(t == 0),
                    stop=(t == CT - 1),
                )
            nc.scalar.activation(uts[:], psU, Act.Relu)

            # s^T per channel tile: sigmoid(w2^T-slice @ uT)  [128, N]
            for t in range(CT):
                nc.tensor.matmul(
                    psS[t],
                    w2s[:, t * 128 : (t + 1) * 128],
                    uts[:],
                    start=True,
                    stop=True,
                )
                nc.scalar.activation(sct[t][:], psS[t], Act.Sigmoid)

            # ---- per-(channel, sample) affine constants ----
            for t in range(CT):
                nc.vector.tensor_scalar(
                    Ac[t][:], inv_b, gw[:, t : t + 1], None, Alu.mult
                )
                nc.vector.tensor_tensor(t8b[:], mu_b, Ac[t][:], Alu.mult)
                nc.vector.tensor_scalar(
                    Bc[t][:], t8b[:], -1.0, gb[:, t : t + 1], Alu.mult, Alu.add
                )
                if t > 0:
                    # folded: out = (x*(1+A) + B) * s  ==  x*sF + bF
                    nc.vector.tensor_scalar(t8a[:], Ac[t][:], 1.0, None, Alu.add)
                    nc.vector.tensor_tensor(sF[t][:], t8a[:], sct[t][:], Alu.mult)
                    nc.vector.tensor_tensor(bF[t][:], Bc[t][:], sct[t][:], Alu.mult)

            # ct0 folded constants: out = x*s + pct*(A*s) + B*s
            As0 = spool.tile([128, 8], fp32, tag="As0", name="As0")
            Bs0 = spool.tile([128, 8], fp32, tag="Bs0", name="Bs0")
            nc.vector.tensor_tensor(As0[:], Ac[0][:], sct[0][:], Alu.mult)
            nc.vector.tensor_tensor(Bs0[:], Bc[0][:], sct[0][:], Alu.mult)

            # ---- fused output pass ----
            for j in range(N):
                nc.vector.tensor_scalar(
                    xt[0, j][:], xt[0, j][:], sct[0][:, j : j + 1], None, Alu.mult
                )
                nc.vector.tensor_scalar(
                    pct[j][:],
                    pct[j][:],
                    As0[:, j : j + 1],
                    Bs0[:, j : j + 1],
                    Alu.mult,
                    Alu.add,
                )
                nc.vector.tensor_tensor(xt[0, j][:], xt[0, j][:], pct[j][:], Alu.add)
                nc.sync.dma_start(
                    outd[j, 0:128].rearrange("c h w -> c (h w)"), xt[0, j][:]
                )
                for t in (1, 2):
                    nc.scalar.activation(
                        xt[t, j][:],
                        xt[t, j][:],
                        Act.Identity,
                        scale=sF[t][:, j : j + 1],
                        bias=bF[t][:, j : j + 1],
                    )
                    nc.sync.dma_start(
                        outd[j, t * 128 : (t + 1) * 128].rearrange("c h w -> c (h w)"),
                        xt[t, j][:],
                    )

    nc.compile()
    return nc


def _get_nc():
    if "nc" not in _compiled:
        _compiled["nc"] = _build()
    return _compiled["nc"]


def run_sharded(inputs, trace=False):
    """inputs: dict of full-size numpy arrays. Returns (full_out, BassKernelResults)."""
    import concourse.bass_utils as bass_utils

    nc = _get_nc()
    x = np.ascontiguousarray(np.asarray(inputs["x"], dtype=np.float32))
    gnw = np.asarray(inputs["gn_weight"], dtype=np.float32)
    gnb = np.asarray(inputs["gn_bias"], dtype=np.float32)
    w1 = np.ascontiguousarray(np.asarray(inputs["w1"], dtype=np.float32))
    w2 = np.ascontiguousarray(np.asarray(inputs["w2"], dtype=np.float32))

    import ml_dtypes

    bands = np.zeros((128, 480), dtype=ml_dtypes.bfloat16)
    k = np.arange(128)
    bands[k[k % 32 < 16], 112 + (k % 32)[k % 32 < 16]] = 1
    bands[k[k % 32 >= 16], 240 + 96 + (k % 32)[k % 32 >= 16]] = 1

    in_maps = []
    for c in range(NCORES):
        shard = np.ascontiguousarray(x[:, :, c * HS : (c + 1) * HS, :])
        in_maps.append(
            {"x": shard, "gnw": gnw, "gnb": gnb, "w1": w1, "w2": w2, "bands": bands}
        )

    res = bass_utils.run_bass_kernel_spmd(
        nc, in_maps, core_ids=list(range(NCORES)), trace=trace
    )
    out = np.empty((N, C, H, W), dtype=np.float32)
    for c in range(NCORES):
        out[:, :, c * HS : (c + 1) * HS, :] = res.results[c]["out"]
    return out, res


def kernel(x, gn_weight, gn_bias, w1, w2):
    out, _ = run_sharded(
        {"x": x, "gn_weight": gn_weight, "gn_bias": gn_bias, "w1": w1, "w2": w2}
    )
    return out

